# revision 2
# baseline (speedup 1.0000x reference)
"""Trainium2 Bass kernel for DigitConvolutionalModel.

Computes, for x [32768, 784] viewed as 28x28 images:
    feat = relu(conv3x3_valid(x))      # [B, 676]
    out  = feat @ W + b                # [B, 10]

Strategy (pure data parallel over 8 cores, 4096 rows each):
  - Host pre-transposes each core's shard to pixel-major and casts to
    bf16, so HBM traffic is halved (6.4 MB/core) and the contraction
    dims sit on SBUF partitions (TensorE contracts partitions only).
  - HBM layout is 4 super-chunks of 1024 batch columns: each DMA line
    is 2 KB (1024 bf16), the best DMA efficiency point, and descriptor
    count is half of a 512-col layout.
  - The 3x3 conv is a banded matmul y^T = C^T @ x^T using two constant
    blocks C1/C2 [112, 128] built on host from conv_w: input rows are
    tiled 4 image rows (112 pixels) per partition group, output rows
    4 conv rows (104 pixels, padded to 128) per PSUM tile.
  - ReLU evacuates PSUM -> SBUF bf16 (split between ScalarE and VectorE).
  - The 676->10 linear layer contracts the same pixel tiles against
    host-packed W blocks, accumulating out^T [10, chunk] fp32 in PSUM;
    bias is added during the PSUM->SBUF copy.
  - Device emits out^T [10, 4096]; host transposes back.
  - PE warm-up: ~3.4us of junk matmuls at program start lift the HAM
    clock-gate to 2.4 GHz before the first real conv matmul.

Walrus accepts only ONE semaphore wait per engine instruction, so the
kernel is arranged so every instruction needs at most one: constants are
pre-touched by tiny warm-up ops, each chunk's x-tile DMA is absorbed by
a touch matmul, redundant same-engine waits Tile emits are stripped, and
the kernel-tail drain is split into single-wait drains.
"""

import numpy as np

try:
    from concourse import bass, mybir
    from concourse.tile import TileContext
    from concourse.bass_utils import run_bass_kernel_spmd
except ImportError:  # path used when concourse is not already importable
    import sys

    sys.path.insert(0, "/opt/trn_rl_repo")
    from concourse import bass, mybir
    from concourse.tile import TileContext
    from concourse.bass_utils import run_bass_kernel_spmd

from concourse.vector_clock import ScopedClock


def _patched_drain_and_barrier(self, tick_clock, wait_clock):
    """Replacement for TileContext._drain_and_barrier: walrus rejects
    instructions carrying more than one sync wait, but the kernel-tail
    drain aggregates a wait per logical proc (~14 here). Emit a chain of
    single-wait drains on the sync queue instead."""
    nc = self.nc
    drain_inst = nc.sync.drain()
    wait_clock.add_sem_waits(
        drain_inst.ins, ScopedClock({None: tick_clock.global_clock})
    )
    si = drain_inst.ins.sync_info
    waits = list(si.on_wait or []) if si else []
    if len(waits) > 1:
        drain_inst.ins.sync_info = mybir.SyncInfo(
            on_wait=waits[:1], on_update=si.on_update
        )
        for w in waits[1:]:
            extra = nc.sync.drain()
            esi = extra.ins.sync_info
            extra.ins.sync_info = mybir.SyncInfo(
                on_wait=[w], on_update=(esi.on_update if esi else [])
            )
    nc.all_engine_barrier()
    popped = nc._tile_sem_poison_stack.pop()
    assert popped == self._sem_poison
    nc.clear_and_free_semaphores(list(self.sems.allocated().values()))
    nc.all_engine_barrier()


TileContext._drain_and_barrier = _patched_drain_and_barrier

N_CORES = 8
B = 32768
B_CORE = B // N_CORES  # 4096
N_SUPER = 4  # super-chunks of 1024 batch columns (2KB bf16 DMA lines)
SUPER = 1024
N_CHUNK = 8  # compute chunks of 512 (PSUM fp32 bank limit)
CHUNK = 512
NT = 7  # pixel-group tiles of 4 image rows (112 pixels); 7*4 = 28 rows

F32 = mybir.dt.float32
BF16 = mybir.dt.bfloat16
RELU = mybir.ActivationFunctionType.Relu
IDENT = mybir.ActivationFunctionType.Identity

_NC_CACHE = {}


def _build_nc():
    nc = bass.Bass(
        "TRN2", target_bir_lowering=False, debug=False, num_devices=1
    )

    # super-chunk-major pixel-major bf16 input: super-chunk S occupies rows
    # 784*S..784*S+783 (row within = pixel), cols = batch within super-chunk
    # - each is one dense 1.6 MB block so the HBM read stream stays
    # sequential with 2KB lines.
    xh = nc.dram_tensor("xh", [N_SUPER * 784, SUPER], BF16, kind="ExternalInput")
    # packed constants: c1 | c2 | wp (columns 0:128 | 128:256 | 256:326)
    cpk_d = nc.dram_tensor("cpk", [128, 326], BF16, kind="ExternalInput")
    bias_d = nc.dram_tensor("bias_in", [10, 1], F32, kind="ExternalInput")
    out_t = nc.dram_tensor("out_t", [10, B_CORE], F32, kind="ExternalOutput")

    with TileContext(nc) as tc:
        with (
            tc.tile_pool(name="const", bufs=1) as cpool,
            tc.tile_pool(name="xc", bufs=1) as xpool,
            tc.tile_pool(name="ry_a", bufs=4) as rypool_a,
            tc.tile_pool(name="ry_v", bufs=4) as rypool_v,
            tc.tile_pool(name="outT", bufs=1) as opool,
            tc.tile_pool(name="yps_a", bufs=2, space="PSUM") as ypool_a,
            tc.tile_pool(name="yps_v", bufs=2, space="PSUM") as ypool_v,
            tc.tile_pool(name="warmp", bufs=2, space="PSUM") as warmpool,
            tc.tile_pool(name="opsum", bufs=2, space="PSUM") as opsum,
        ):
            # splits[S]: pixel-block boundaries for super-chunk S's DMAs.
            # The first super-chunk loads in pieces so the conv pipeline
            # starts as soon as the first blocks land.
            splits = {0: (0, 1, 2, 4, NT), 1: (0, 4, NT)}

            def load_block(tile, S, lo, hi):
                blk = bass.AP(
                    xh,
                    (784 * S + 112 * lo) * SUPER,
                    [[SUPER, 112], [112 * SUPER, hi - lo], [1, SUPER]],
                )
                nc.gpsimd.dma_start(tile[:, SUPER * lo : SUPER * hi], blk)

            def make_tile(S):
                return xpool.tile(
                    [112, NT * SUPER], BF16, tag=f"xc{S}", name=f"xc{S}"
                )

            # x block 0 of super-chunk 0 goes first (it gates the first
            # conv matmul), then the packed constants, then the rest.
            xc0 = make_tile(0)
            load_block(xc0, 0, 0, 1)
            cpk_sb = cpool.tile([128, 326], BF16, tag="cpk")
            nc.gpsimd.dma_start(cpk_sb[:], cpk_d.ap())
            c1_sb = cpk_sb[0:112, 0:128]
            c2_sb = cpk_sb[0:112, 128:256]
            wp_sb = cpk_sb[:, 256:326]
            bias_sb = cpool.tile([10, 1], F32, tag="bias")
            nc.gpsimd.dma_start(bias_sb[:], bias_d.ap())
            for lo, hi in zip(splits[0][1:], splits[0][2:]):
                load_block(xc0, 0, lo, hi)
            xc = [xc0]
            for S in range(1, N_SUPER):
                tile = make_tile(S)
                for lo, hi in zip(
                    splits.get(S, (0, NT)), splits.get(S, (0, NT))[1:]
                ):
                    load_block(tile, S, lo, hi)
                xc.append(tile)

            outT_sb = opool.tile([10, B_CORE], F32, tag="outT")

            # PE HAM warm-up: the PE clock-gate only lifts to 2.4 GHz after
            # ~3us of sustained activity. Fill the initial DMA-wait window
            # with junk matmuls so the real matmuls run warm. The memset
            # runs on the otherwise-idle VectorE (the Pool queue is busy
            # issuing DMA triggers).
            junk = cpool.tile([112, 512], BF16, tag="junk")
            nc.vector.memset(junk[:], 0.0)
            warm = warmpool.tile([8, 512], F32, tag="warm")
            warm2 = warmpool.tile([8, 512], F32, tag="warm")
            # high_priority pins the spam at the head of the PE stream.
            with tc.high_priority():
                for i in range(8):
                    nc.tensor.matmul(
                        (warm if i % 2 == 0 else warm2)[:],
                        junk[:, 0:8],
                        junk[:],
                    )

            # Pre-touch the constants with a tiny op so real instructions'
            # dependency on their DMA is satisfied by engine program order
            # (walrus allows a single sync wait per instruction).
            nc.tensor.matmul(warm[0:4, 0:4], c1_sb[:, 0:4], c1_sb[:, 0:4])
            warm_act = cpool.tile([10, 1], F32, tag="warm_act")
            nc.scalar.activation(warm_act[:], bias_sb[:], IDENT, bias=bias_sb[:])

            for n in range(N_CHUNK):
                S, half = divmod(n, 2)
                coff = SUPER * 0 + half * CHUNK  # column offset inside tile

                def xcol(t, lo=0, hi=CHUNK):
                    return xc[S][:, SUPER * t + half * CHUNK + lo : SUPER * t + half * CHUNK + hi]

                # Touch matmuls absorb this super-chunk's DMA waits on PE so
                # the conv matmuls only carry their PSUM-slot wait.
                if half == 0:
                    for lo in splits.get(S, (0, NT))[:-1]:
                        nc.tensor.matmul(
                            warm[0:4, 0:4],
                            xc[S][:, SUPER * lo : SUPER * lo + 4],
                            xc[S][:, SUPER * lo : SUPER * lo + 4],
                        )
                rys = []
                for t in range(NT):
                    on_act = t % 2 == 0
                    yps = (ypool_a if on_act else ypool_v).tile(
                        [128, CHUNK], F32, tag="yps"
                    )
                    nc.tensor.matmul(
                        yps[:],
                        c1_sb,
                        xcol(t),
                        start=True,
                        stop=(t == 6),
                    )
                    if t < 6:
                        nc.tensor.matmul(
                            yps[:],
                            c2_sb,
                            xcol(t + 1),
                            start=False,
                            stop=True,
                        )
                    ry = (rypool_a if on_act else rypool_v).tile(
                        [128, CHUNK], BF16, tag="ry"
                    )
                    if on_act:
                        nc.scalar.activation(ry[:], yps[:], RELU)
                    else:
                        nc.vector.tensor_relu(ry[:], yps[:])
                    rys.append(ry)

                ops = opsum.tile([10, CHUNK], F32, tag="ops")
                for t in range(NT):
                    nc.tensor.matmul(
                        ops[:],
                        wp_sb[:, 10 * t : 10 * (t + 1)],
                        rys[t][:],
                        start=(t == 0),
                        stop=(t == 6),
                    )
                nc.scalar.activation(
                    outT_sb[:, CHUNK * n : CHUNK * (n + 1)],
                    ops[:],
                    IDENT,
                    bias=bias_sb[:],
                )
                # Output DMAs on the otherwise-idle SP queue: writing as
                # compute finishes hides the HBM write-receipt latency of
                # all but the last chunk. Merged to 8 DMAs total so each
                # gets a fresh DMA lane (single-wait trigger).
                nc.sync.dma_start(
                    out_t.ap()[:, CHUNK * n : CHUNK * (n + 1)],
                    outT_sb[:, CHUNK * n : CHUNK * (n + 1)],
                )

    _strip_self_waits(nc)
    return nc


_ENGINE_SEM_PREFIX = {
    mybir.EngineType.PE: "PE_",
    mybir.EngineType.Activation: "Activation_",
    mybir.EngineType.DVE: "DVE_",
    mybir.EngineType.Pool: "Pool_",
    mybir.EngineType.SP: "SP_",
}


def _strip_self_waits(nc):
    """Drop semaphore waits an instruction holds on its OWN engine's
    completion counter. Engines execute their queue strictly in order, so
    a wait on the own-engine sem at a value covered by program order is
    redundant — but Tile still emits it, and walrus rejects compute
    instructions carrying more than one sync wait."""
    for fn in nc.m.functions:
        for blk in fn.blocks:
            for inst in blk.instructions:
                tn = type(inst).__name__
                if tn in ("InstDrain", "InstEventSemaphore", "InstDMACopy"):
                    continue
                si = inst.sync_info
                if si is None or not si.on_wait or len(si.on_wait) < 2:
                    continue
                pref = _ENGINE_SEM_PREFIX.get(inst.engine)
                if pref is None:
                    continue
                kept = [w for w in si.on_wait if not w.ant_name.startswith(pref)]
                if len(kept) != len(si.on_wait):
                    inst.sync_info = mybir.SyncInfo(
                        on_wait=kept, on_update=si.on_update
                    )


def _build_consts(conv_w, W, b):
    conv_w = np.asarray(conv_w, np.float32)
    W = np.asarray(W, np.float32)
    b = np.asarray(b, np.float32)

    # C1: input rows 4t+rl (rl 0..3) -> output conv rows 4t+il (il 0..3)
    # C2: input rows 4(t+1)+rl      -> output conv rows 4t+il
    c1 = np.zeros((112, 128), np.float32)
    c2 = np.zeros((112, 128), np.float32)
    for rl in range(4):
        for c in range(28):
            for il in range(4):
                for j in range(26):
                    dj = c - j
                    if not (0 <= dj <= 2):
                        continue
                    di = rl - il
                    if 0 <= di <= 2:
                        c1[rl * 28 + c, il * 26 + j] = conv_w[di, dj]
                    di2 = 4 + rl - il
                    if 0 <= di2 <= 2:
                        c2[rl * 28 + c, il * 26 + j] = conv_w[di2, dj]

    # W packed: block t holds rows for conv-output rows 4t..4t+3
    wp = np.zeros((128, 70), np.float32)
    for t in range(6):
        wp[0:104, 10 * t : 10 * (t + 1)] = W[104 * t : 104 * (t + 1)]
    wp[0:52, 60:70] = W[624:676]

    import ml_dtypes

    cpk = np.zeros((128, 326), np.float32)
    cpk[0:112, 0:128] = c1
    cpk[0:112, 128:256] = c2
    cpk[:, 256:326] = wp
    return cpk.astype(ml_dtypes.bfloat16), b.reshape(10, 1).copy()


def _run(inputs, trace=False):
    import ml_dtypes

    x = np.asarray(inputs["x"], np.float32)
    conv_w = inputs["conv_w"]
    W = inputs["W"]
    b = inputs["b"]

    if "nc" not in _NC_CACHE:
        _NC_CACHE["nc"] = _build_nc()
    nc = _NC_CACHE["nc"]

    cpk, bias = _build_consts(conv_w, W, b)

    xb = x.astype(ml_dtypes.bfloat16)
    in_maps = []
    for c in range(N_CORES):
        shard = xb[c * B_CORE : (c + 1) * B_CORE]  # [4096, 784] bf16
        # [4, 1024, 784] -> [4, 784, 1024]: super-chunk-major, pixel rows
        xh = np.ascontiguousarray(
            shard.reshape(N_SUPER, SUPER, 784).transpose(0, 2, 1)
        ).reshape(N_SUPER * 784, SUPER)
        in_maps.append({"xh": xh, "cpk": cpk, "bias_in": bias})

    res = run_bass_kernel_spmd(
        nc, in_maps, core_ids=list(range(N_CORES)), trace=trace
    )
    out = np.concatenate(
        [np.asarray(res.results[c]["out_t"]).T for c in range(N_CORES)], axis=0
    )
    return out, res


def kernel(**inputs) -> np.ndarray:
    return _run(inputs, trace=False)[0]


# revision 16
# speedup vs baseline: 1.1614x; 1.1614x over previous
"""Trainium2 Bass kernel for DigitConvolutionalModel.

Computes, for x [32768, 784] viewed as 28x28 images:
    feat = relu(conv3x3_valid(x))      # [B, 676]
    out  = feat @ W + b                # [B, 10]

Strategy (pure data parallel over 8 cores, 4096 rows each):
  - Host pre-transposes each core's shard to pixel-major and casts to
    bf16 (HBM traffic halves to 6.4 MB/core); contraction dims sit on
    SBUF partitions.
  - HBM layout is 4 super-chunks of 1024 batch columns -> 2 KB DMA
    lines; 16 roughly equal SWDGE loads keep all 16 DMA queues busy.
  - The conv runs in the PE's 128x64 column-tiled mode: output tiles of
    2 conv rows (52 px, padded to 64 partitions) are assigned to PSUM
    partition slices [0:64] (A) / [64:128] (B). MMs targeting different
    slices execute CONCURRENTLY in the two column halves of the PE
    array, so the 19 conv MMs + 7 linear MMs per 512-col chunk cost
    ~13 MM slots instead of 26.
  - Three host-built stationaries: C_E [112,64] (even tiles, one block),
    C_O1/C_O2 (odd tiles, split over two adjacent blocks), each padded
    with zero columns to 64 so PSUM pad partitions read 0, never NaN.
  - ReLU evacuates each PSUM bank (a pair of output tiles) to bf16 SBUF,
    split over ScalarE + VectorE + GpSimd.
  - The 676->10 linear contracts ry banks against wp blocks [128,64]
    (10 used cols), A-chain -> LA[0:10], B-chain -> LB[64:74]; a single
    DVE scalar_tensor_tensor computes (LA + bias) + LB per chunk.
  - Device emits out^T [10, 4096]; host transposes back.
  - PE HAM warm-up: junk matmuls (same 128x64 mode) fill the initial
    DMA wait so real matmuls run at 2.4 GHz.

Walrus accepts only ONE semaphore wait per engine instruction, so the
kernel is arranged so every instruction needs at most one: constants are
pre-touched, x-block DMA waits are absorbed by touch matmuls interleaved
with the conv slots, relu buffer reuse pairs with same-engine sems, and
the kernel-tail drain is split into single-wait drains.
"""

import numpy as np

try:
    from concourse import bass, mybir
    from concourse.tile import TileContext
    from concourse.bass_utils import run_bass_kernel_spmd
except ImportError:  # path used when concourse is not already importable
    import sys

    sys.path.insert(0, "/opt/trn_rl_repo")
    from concourse import bass, mybir
    from concourse.tile import TileContext
    from concourse.bass_utils import run_bass_kernel_spmd

from concourse.vector_clock import ScopedClock


def _patched_drain_and_barrier(self, tick_clock, wait_clock):
    """Replacement for TileContext._drain_and_barrier: walrus rejects
    instructions carrying more than one sync wait, but the kernel-tail
    drain aggregates a wait per logical proc. Emit a chain of
    single-wait drains on the sync queue instead."""
    nc = self.nc
    drain_inst = nc.sync.drain()
    wait_clock.add_sem_waits(
        drain_inst.ins, ScopedClock({None: tick_clock.global_clock})
    )
    si = drain_inst.ins.sync_info
    waits = list(si.on_wait or []) if si else []
    if len(waits) > 1:
        drain_inst.ins.sync_info = mybir.SyncInfo(
            on_wait=waits[:1], on_update=si.on_update
        )
        for w in waits[1:]:
            extra = nc.sync.drain()
            esi = extra.ins.sync_info
            extra.ins.sync_info = mybir.SyncInfo(
                on_wait=[w], on_update=(esi.on_update if esi else [])
            )
    nc.all_engine_barrier()
    popped = nc._tile_sem_poison_stack.pop()
    assert popped == self._sem_poison
    nc.clear_and_free_semaphores(list(self.sems.allocated().values()))
    nc.all_engine_barrier()


TileContext._drain_and_barrier = _patched_drain_and_barrier

N_CORES = 8
B = 32768
B_CORE = B // N_CORES  # 4096
N_SUPER = 4  # super-chunks of 1024 batch columns (2KB bf16 DMA lines)
SUPER = 1024
N_CHUNK = 8  # compute chunks of 512 (PSUM fp32 bank limit)
CHUNK = 512
NT = 7  # pixel blocks of 4 image rows (112 pixels); 7*4 = 28 rows

F32 = mybir.dt.float32
BF16 = mybir.dt.bfloat16
RELU = mybir.ActivationFunctionType.Relu
IDENT = mybir.ActivationFunctionType.Identity
ADD = mybir.AluOpType.add

# ---- conv tiling tables ------------------------------------------------
# Output tile s covers conv-output rows (2s, 2s+1), s = 0..12 (26 rows).
# Even s: one MM, C_E (x) block s//2.  Odd s: C_O1 (x) block (s-1)//2 then
# C_O2 (x) block (s+1)//2, accumulating.
# PSUM banks hold a pair (A-tile in partitions [0:64], B-tile [64:128]):
PAIRS = [(0, 1), (3, 2), (4, 5), (7, 6), (8, 9), (11, 10), (12, None)]
# relu engine per bank: 'v' = DVE, 'a' = ScalarE (GpSimd cannot read PSUM).
# ScalarE also does both bias-add evacuations, so DVE takes 4 relus.
RELU_ENG = ["v", "v", "a", "a", "v", "v", "a"]
# linear contraction order: both chains start with an Act-relu'd bank so
# the first matmul's PSUM-reuse (Act) and relu (Act) waits merge into one.
LIN_ORDER_A = [2, 0, 1, 3, 4, 5, 6]
LIN_ORDER_B = [3, 0, 1, 2, 4, 5, 6]
# The linear is split by OUTPUT COLUMNS: the A chain (PSUM partitions
# [0:64], PE column half 0) computes output cols [0:256], the B chain
# ([64:128], half 1) cols [256:512]. Both contract all 7 ry banks with
# 256-col matmuls, so the chains run concurrently and each produces
# final sums - no cross-chain partial add is needed.

_NC_CACHE = {}


def _build_nc(n_warm=8):
    nc = bass.Bass(
        "TRN2", target_bir_lowering=False, debug=False, num_devices=1
    )

    # super-chunk-major pixel-major bf16 input: super-chunk S occupies rows
    # 784*S..784*S+783 (row = pixel), cols = batch within super-chunk.
    xh = nc.dram_tensor("xh", [N_SUPER * 784, SUPER], BF16, kind="ExternalInput")
    # packed constants: C_E | C_O1 | C_O2 | wp0..wp6  (64 cols each)
    cpk_d = nc.dram_tensor("cpk", [128, 640], BF16, kind="ExternalInput")
    bias_d = nc.dram_tensor("bias_in", [10, 1], F32, kind="ExternalInput")
    out_t = nc.dram_tensor("out_t", [10, B_CORE], F32, kind="ExternalOutput")

    with TileContext(nc) as tc:
        with (
            tc.tile_pool(name="const", bufs=1) as cpool,
            tc.tile_pool(name="xc", bufs=1) as xpool,
            tc.tile_pool(name="ry_a", bufs=3) as rypool_a,
            tc.tile_pool(name="ry_v", bufs=3) as rypool_v,
            tc.tile_pool(name="ry_p", bufs=3) as rypool_p,
            tc.tile_pool(name="outT", bufs=1) as opool,
            tc.tile_pool(name="yps_a", bufs=2, space="PSUM") as ypool_a,
            tc.tile_pool(name="yps_v", bufs=2, space="PSUM") as ypool_v,
            tc.tile_pool(name="warmp", bufs=2, space="PSUM") as warmpool,
            tc.tile_pool(name="lpsA", bufs=1, space="PSUM") as lpool_a,
            tc.tile_pool(name="lpsB", bufs=1, space="PSUM") as lpool_b,
        ):
            # per-super-chunk DMA split points (block granularity). Super 0
            # loads per block so the conv pipeline starts on block 0; the
            # rest load in 2-block (448 KB) pieces -> 16 balanced queues.
            splits = {0: (0, 1, 2, 3, 4, 5, 6, 7)}
            SPLIT_REST = (0, 2, 4, 6, 7)

            def load_block(tile, S, lo, hi):
                blk = bass.AP(
                    xh,
                    (784 * S + 112 * lo) * SUPER,
                    [[SUPER, 112], [112 * SUPER, hi - lo], [1, SUPER]],
                )
                nc.gpsimd.dma_start(tile[:, SUPER * lo : SUPER * hi], blk)

            def make_tile(S):
                return xpool.tile(
                    [112, NT * SUPER], BF16, tag=f"xc{S}", name=f"xc{S}"
                )

            # x block 0 first (it gates the first conv matmul), then the
            # packed constants, then everything else.
            xc0 = make_tile(0)
            load_block(xc0, 0, 0, 1)
            cpk_sb = cpool.tile([128, 640], BF16, tag="cpk")
            nc.gpsimd.dma_start(cpk_sb[:], cpk_d.ap())
            ce_sb = cpk_sb[0:112, 0:64]
            co1_sb = cpk_sb[0:112, 64:128]
            co2_sb = cpk_sb[0:112, 128:192]
            wp_sb = [cpk_sb[:, 192 + 64 * k : 256 + 64 * k] for k in range(7)]
            bias_sb = cpool.tile([10, 1], F32, tag="bias")
            nc.gpsimd.dma_start(bias_sb[:], bias_d.ap())
            for lo, hi in zip(splits[0][1:], splits[0][2:]):
                load_block(xc0, 0, lo, hi)
            xc = [xc0]
            for S in range(1, N_SUPER):
                tile = make_tile(S)
                sp = splits.get(S, SPLIT_REST)
                for lo, hi in zip(sp, sp[1:]):
                    load_block(tile, S, lo, hi)
                xc.append(tile)

            outT_sb = opool.tile([10, B_CORE], F32, tag="outT")

            # ry7's B half is never written (bank 7 holds only tile 12);
            # zero it once so the linear matmul contracts 0s, not NaNs.
            ry7 = cpool.tile([128, CHUNK], BF16, tag="ry7")
            nc.vector.memset(ry7[64:128, :], 0.0)
            # DVE pre-touch of bias (used as the scalar AP in the per-chunk
            # scalar_tensor_tensor) so that op carries only its PE wait.
            bias_touch = cpool.tile([10, 1], F32, tag="bias_touch")
            nc.vector.tensor_scalar_add(bias_touch[:], bias_sb[:], 0.0)

            # PE HAM warm-up in the same 128x64 tile mode as everything
            # else (mode switches drain the PE). Junk tile is zeroed by
            # the otherwise-idle ScalarE.
            junk = cpool.tile([112, 512], BF16, tag="junk")
            nc.scalar.memzero(junk[:])
            warm = warmpool.tile([128, 512], F32, tag="warm")
            warm2 = warmpool.tile([128, 512], F32, tag="warm")
            with tc.high_priority():
                for i in range(n_warm):
                    nc.tensor.matmul(
                        (warm if i % 2 == 0 else warm2)[0:64, :],
                        junk[:, 0:64],
                        junk[:],
                    )

            # Pre-touch cpk on PE (one tiny matmul covers the whole tile's
            # DMA sem; later readers rely on PE program order).
            nc.tensor.matmul(warm[0:64, 0:4], ce_sb, junk[:, 0:4])
            warm_act = cpool.tile([10, 1], F32, tag="warm_act")
            nc.scalar.activation(warm_act[:], bias_sb[:], IDENT, bias=bias_sb[:])

            def touch(S, lo):
                # absorb the DMA-completion wait of x load (S, lo..) on PE
                nc.tensor.matmul(
                    warm[0:64, 0:4],
                    xc[S][:, SUPER * lo : SUPER * lo + 64],
                    xc[S][:, SUPER * lo : SUPER * lo + 4],
                )

            for n in range(N_CHUNK):
                S, half = divmod(n, 2)
                sp = splits.get(S, SPLIT_REST)

                def xcol(t):
                    o = SUPER * t + half * CHUNK
                    return xc[S][:, o : o + CHUNK]

                # conv MMs for this chunk, in (emission order) with the
                # DMA-wait touches interleaved right before the first slot
                # that needs each newly-split block.
                ypsum = {}
                done_mm = {}  # bank -> remaining MM count

                def conv_mm(bank, side, cmat, t, start, stop):
                    yps = ypsum[bank]
                    sl = yps[0:64, :] if side == "A" else yps[64:128, :]
                    nc.tensor.matmul(sl, cmat, xcol(t), start=start, stop=stop)

                def relu(bank):
                    yps = ypsum[bank]
                    eng = RELU_ENG[bank]
                    if bank == 6:
                        ry = ry7
                        nc.scalar.activation(ry[0:64, :], yps[0:64, :], RELU)
                        return ry
                    pool = {"a": rypool_a, "v": rypool_v, "p": rypool_p}[eng]
                    ry = pool.tile(
                        [128, CHUNK], BF16, tag=f"ry{eng}", name=f"ry{eng}"
                    )
                    if eng == "a":
                        nc.scalar.activation(ry[:], yps[:], RELU)
                    elif eng == "v":
                        nc.vector.tensor_relu(ry[:], yps[:])
                    else:
                        nc.gpsimd.tensor_relu(ry[:], yps[:])
                    return ry

                if half == 0 and n > 0:
                    # blocks 0..1 of this super-chunk must be resident
                    # before bank 0's MMs; interleaved touches below cover
                    # later splits.
                    pass

                # bank allocation helper
                def bk(i):
                    return (ypool_a if i % 2 == 0 else ypool_v).tile(
                        [128, CHUNK], F32, tag="yps", name=f"yps{i}"
                    )

                rys = {}
                tch = list(sp[:-1]) if half == 0 else []
                tix = 0

                def maybe_touch(need_blk):
                    nonlocal tix
                    while tix < len(tch) and tch[tix] <= need_blk:
                        touch(S, tch[tix])
                        tix += 1

                # --- emission schedule (A/B alternate for concurrency) ---
                maybe_touch(0)
                ypsum[0] = bk(0)
                conv_mm(0, "A", ce_sb, 0, True, True)        # M0  tile0
                conv_mm(0, "B", co1_sb, 0, True, False)      # M1a tile1
                maybe_touch(1)
                ypsum[1] = bk(1)
                conv_mm(0, "B", co2_sb, 1, False, True)      # M1b tile1
                conv_mm(1, "A", co1_sb, 1, True, False)      # M3a tile3
                rys[0] = relu(0)
                maybe_touch(2)
                conv_mm(1, "B", ce_sb, 1, True, True)        # M2  tile2
                conv_mm(1, "A", co2_sb, 2, False, True)      # M3b tile3
                ypsum[2] = bk(2)
                rys[1] = relu(1)
                conv_mm(2, "A", ce_sb, 2, True, True)        # M4  tile4
                conv_mm(2, "B", co1_sb, 2, True, False)      # M5a tile5
                maybe_touch(3)
                ypsum[3] = bk(3)
                conv_mm(2, "B", co2_sb, 3, False, True)      # M5b tile5
                conv_mm(3, "A", co1_sb, 3, True, False)      # M7a tile7
                rys[2] = relu(2)
                maybe_touch(4)
                conv_mm(3, "B", ce_sb, 3, True, True)        # M6  tile6
                conv_mm(3, "A", co2_sb, 4, False, True)      # M7b tile7
                ypsum[4] = bk(4)
                rys[3] = relu(3)
                conv_mm(4, "A", ce_sb, 4, True, True)        # M8  tile8
                conv_mm(4, "B", co1_sb, 4, True, False)      # M9a tile9
                maybe_touch(5)
                ypsum[5] = bk(5)
                conv_mm(4, "B", co2_sb, 5, False, True)      # M9b tile9
                conv_mm(5, "A", co1_sb, 5, True, False)      # M11a tile11
                rys[4] = relu(4)
                maybe_touch(6)
                conv_mm(5, "B", ce_sb, 5, True, True)        # M10 tile10
                conv_mm(5, "A", co2_sb, 6, False, True)      # M11b tile11
                ypsum[6] = bk(6)
                rys[5] = relu(5)
                conv_mm(6, "A", ce_sb, 6, True, True)        # M12 tile12
                rys[6] = relu(6)

                # --- linear: A chain -> out cols [0:256] in la_a[0:64],
                #             B chain -> out cols [256:512] in la_v[64:128];
                # the chains run in the two PE column halves concurrently
                # and each produces final sums for its column half.
                HC = CHUNK // 2
                la_a = lpool_a.tile([128, CHUNK], F32, tag="la_a")
                la_v = lpool_b.tile([128, CHUNK], F32, tag="la_v")
                for i in range(7):
                    ka, kb = LIN_ORDER_A[i], LIN_ORDER_B[i]
                    nc.tensor.matmul(
                        la_a[0:64, 0:HC], wp_sb[ka], rys[ka][:, 0:HC],
                        start=(i == 0), stop=(i == 6),
                    )
                    nc.tensor.matmul(
                        la_v[64:128, HC:CHUNK], wp_sb[kb], rys[kb][:, HC:CHUNK],
                        start=(i == 0), stop=(i == 6),
                    )

                # bias-add evacuation, both halves on ScalarE so the single
                # out-DMA carries one producer wait.
                nc.scalar.activation(
                    outT_sb[:, CHUNK * n : CHUNK * n + HC],
                    la_a[0:10, 0:HC],
                    IDENT,
                    bias=bias_sb[:],
                )
                nc.scalar.activation(
                    outT_sb[:, CHUNK * n + HC : CHUNK * (n + 1)],
                    la_v[64:74, HC:CHUNK],
                    IDENT,
                    bias=bias_sb[:],
                )
                nc.sync.dma_start(
                    out_t.ap()[:, CHUNK * n : CHUNK * (n + 1)],
                    outT_sb[:, CHUNK * n : CHUNK * (n + 1)],
                )

    _strip_self_waits(nc)
    return nc


_ENGINE_SEM_PREFIX = {
    mybir.EngineType.PE: "PE_",
    mybir.EngineType.Activation: "Activation_",
    mybir.EngineType.DVE: "DVE_",
    mybir.EngineType.Pool: "Pool_",
    mybir.EngineType.SP: "SP_",
}


def _strip_self_waits(nc):
    """Drop semaphore waits an instruction holds on its OWN engine's
    completion counter (redundant with program order; walrus rejects
    compute instructions carrying more than one sync wait)."""
    for fn in nc.m.functions:
        for blk in fn.blocks:
            for inst in blk.instructions:
                tn = type(inst).__name__
                if tn in ("InstDrain", "InstEventSemaphore", "InstDMACopy"):
                    continue
                si = inst.sync_info
                if si is None or not si.on_wait or len(si.on_wait) < 2:
                    continue
                pref = _ENGINE_SEM_PREFIX.get(inst.engine)
                if pref is None:
                    continue
                kept = [w for w in si.on_wait if not w.ant_name.startswith(pref)]
                if len(kept) != len(si.on_wait):
                    inst.sync_info = mybir.SyncInfo(
                        on_wait=kept, on_update=si.on_update
                    )


def _build_consts(conv_w, W, b):
    conv_w = np.asarray(conv_w, np.float32)
    W = np.asarray(W, np.float32)
    b = np.asarray(b, np.float32)

    # C_E: even tile s=2p: out local row i in {0,1} (rows 4p+i), input
    # local rl (rows 4p+rl): tap d = rl - i.
    # C_O1: odd tile s=2p+1 from block p: out row 4p+2+i: d = rl - 2 - i.
    # C_O2: odd tile from block p+1: input row 4p+4+rl: d = rl + 2 - i.
    ce = np.zeros((112, 64), np.float32)
    co1 = np.zeros((112, 64), np.float32)
    co2 = np.zeros((112, 64), np.float32)
    for rl in range(4):
        for c in range(28):
            p = rl * 28 + c
            for i in range(2):
                for j in range(26):
                    dj = c - j
                    if not (0 <= dj <= 2):
                        continue
                    d = rl - i
                    if 0 <= d <= 2:
                        ce[p, i * 26 + j] = conv_w[d, dj]
                    d = rl - 2 - i
                    if 0 <= d <= 2:
                        co1[p, i * 26 + j] = conv_w[d, dj]
                    d = rl + 2 - i
                    if 0 <= d <= 2:
                        co2[p, i * 26 + j] = conv_w[d, dj]

    # wp_k: linear weights for bank k = pair (sA, sB): W rows for the A
    # tile's 52 pixels at partitions 0:52, B tile's at 64:116.
    wps = np.zeros((128, 7 * 64), np.float32)
    for k, (sa, sb) in enumerate(PAIRS):
        wps[0:52, 64 * k : 64 * k + 10] = W[52 * sa : 52 * sa + 52]
        if sb is not None:
            wps[64:116, 64 * k : 64 * k + 10] = W[52 * sb : 52 * sb + 52]

    import ml_dtypes

    cpk = np.zeros((128, 640), np.float32)
    cpk[0:112, 0:64] = ce
    cpk[0:112, 64:128] = co1
    cpk[0:112, 128:192] = co2
    cpk[:, 192:640] = wps
    return cpk.astype(ml_dtypes.bfloat16), b.reshape(10, 1).copy()


def _run(inputs, trace=False):
    import ml_dtypes

    x = np.asarray(inputs["x"], np.float32)
    conv_w = inputs["conv_w"]
    W = inputs["W"]
    b = inputs["b"]

    if "nc" not in _NC_CACHE:
        _NC_CACHE["nc"] = _build_nc()
    nc = _NC_CACHE["nc"]

    cpk, bias = _build_consts(conv_w, W, b)

    xb = x.astype(ml_dtypes.bfloat16)
    in_maps = []
    for c in range(N_CORES):
        shard = xb[c * B_CORE : (c + 1) * B_CORE]  # [4096, 784] bf16
        xh = np.ascontiguousarray(
            shard.reshape(N_SUPER, SUPER, 784).transpose(0, 2, 1)
        ).reshape(N_SUPER * 784, SUPER)
        in_maps.append({"xh": xh, "cpk": cpk, "bias_in": bias})

    res = run_bass_kernel_spmd(
        nc, in_maps, core_ids=list(range(N_CORES)), trace=trace
    )
    out = np.concatenate(
        [np.asarray(res.results[c]["out_t"]).T for c in range(N_CORES)], axis=0
    )
    return out, res


def kernel(**inputs) -> np.ndarray:
    return _run(inputs, trace=False)[0]
